# revision 35
# baseline (speedup 1.0000x reference)
"""Trainium2 Bass kernel for nn_AttentionBlock (B=4, T=2048, C=K=V=1024).

Self-contained: builds one SPMD Bass/Tile program, runs it on 8 NeuronCores
via run_bass_kernel_spmd, and reassembles the full output on the host.

Math (matches the reference):
  q/k/v = x @ W + b ; logits[b,t,s] = q.k, causal mask s<=t ;
  probs = softmax(logits/sqrt(K), axis=t)   # over the QUERY axis
  read = probs @ v ; out = concat(x, read, axis=2)

Sharding (zero-collective): core = 2*b + h owns batch b and the interleaved
key/value tiles sigma = 2*i + h (interleaving balances the causal triangle).
Because the softmax normalizes over the query axis t and each core has ALL
queries for its own key columns, the softmax is fully core-local. Each core
computes q in full, k/v only for its own columns, exp-normalized scores
et[s_own, t], and an additive partial read_h[t, v] = et^T @ v_own. The host
sums the two partials per batch and concatenates x.

Numerics: the q/k path runs in fp8e4m3 with DoubleRow matmuls (2
contraction rows/cycle): x is pre-scaled x16, Wq/Wk x256 and quantized to
fp8 on the host; phase A/B accumulate in f32 PSUM and write qt/kt as
fp8(q*32) (drain scale 1/128, bias pre-scaled x32). Phase C contracts
qt8.kt8 with DoubleRow; its PSUM holds logits*1024, folded into the exp
scale 1/(32*1024). The v projection (phase D) stays bf16 (fp8 there
busts the 2e-2 gate: v rounding hits the read linearly, while q/k
rounding only perturbs the softmax exponent). Phase E mixes dtypes per
s-block: the near-diagonal block (large p) runs bf16, far blocks run
fp8 DoubleRow (p8 = fp8(p*128), v8 = fp8(v*32)) - their p entries are
tiny, measured ~1e-4 extra relmax. The bf16 leg carries v*4096 so both
legs accumulate p*v*4096 in one PSUM; the drain folds 2^-12 back out.
read_out is written bf16 and upcast on the host. CPU-emulated end-to-end
relmax 1.17e-2 (gate 2e-2); bf16-everywhere measures 2.5e-3 on silicon.

A HAM warmup (20 dummy fp8 matmuls on a zeroed tile) keeps the PE busy
through the input-DMA head so the clock gate is 8/8 when phase A starts.

Phases (single dense PE pipeline, no collectives):
  A. qt8 [k, 2048] = fp8((wq8.T @ xt8)/128 + bq*32)     DoubleRow fp8
     (contraction ci2-outer with 4 live PSUM banks per ko group, so
      the first matmul only waits on ~0.8 MB of DMA, not 3 MB)
  B. kt8_own [k, 1024] likewise                          DoubleRow fp8
  C. et_i = exp((kt8_i.T @ qt8 + mask)/32768), fused row-sum -> dinv;
     et_i *= dinv on DVE (bf16) + fp8 x128 copy on ACT   DoubleRow fp8
  D. v_own [1024, v] = (xst16.T @ wv16 + bv)*4096 (bf16) + fp8 /128 copy
  E. read_partial[t, v] = sum_i et_i.T @ v_own_i -> DRAM (bf16;
     far blocks fp8 DoubleRow, near-diagonal block bf16)
"""

from contextlib import ExitStack

import numpy as np

import concourse.mybir as mybir
import concourse.tile as tile
from concourse import bacc
from concourse._compat import with_exitstack

P = 128
B = 4
T = 2048
C = 1024
KD = 1024
VD = 1024
NCO = C // P
NKO = KD // P
NI = 8
SOWN = NI * P
SQRT_K = 32.0
F32 = mybir.dt.float32
CD = mybir.dt.bfloat16
F8 = mybir.dt.float8e4
# Host-side quantization scales (all powers of two, folded exactly):
#   x8 = fp8(x*XS), w8 = fp8(W*WS), qt8 = q*QS  (via drain scale QS/(XS*WS))
XS = 16.0
WS = 256.0
QS = 32.0
SB = 512


@with_exitstack
def attn_body(ctx: ExitStack, tc, io):
    nc = tc.nc
    DRM = mybir.MatmulPerfMode.DoubleRow
    MUL = mybir.AluOpType.mult
    ADD = mybir.AluOpType.add
    xt8 = io["xt8"].ap().rearrange("(co ci) t -> ci co t", ci=P)
    xst8 = io["xst8"].ap().rearrange("(co ci) t -> ci co t", ci=P)
    xst16 = io["xst16"].ap().rearrange("(co ci) t -> ci co t", ci=P)
    wq8 = io["wq8"].ap().rearrange("(co ci) k -> ci co k", ci=P)
    wk8 = io["wk8"].ap().rearrange("(co ci) k -> ci co k", ci=P)
    wv16 = io["wv16"].ap().rearrange("(co ci) k -> ci co k", ci=P)

    const = ctx.enter_context(tc.tile_pool(name="const", bufs=1))
    bk_sb = const.tile([P, NKO], F32)
    bq_sb = const.tile([P, NKO], F32)
    mask_sb = const.tile([P, 2 * P], F32)
    # const loads ride the scalar-engine DMA queue so they don't delay
    # the critical first wq/xt chunks on the sync/gpsimd queues
    nc.scalar.dma_start(bk_sb[:], io["bk2"].ap())
    nc.scalar.dma_start(bq_sb[:], io["bq2"].ap())
    nc.scalar.dma_start(mask_sb[:], io["maskbias"].ap())

    psum = ctx.enter_context(tc.tile_pool(name="psum", bufs=8, space="PSUM"))

    wp = ctx.enter_context(tc.tile_pool(name="wp", bufs=1, side="right"))
    wq_sb = wp.tile([P, NCO, KD], F8, name="wq_sb")
    wk_sb = wp.tile([P, NCO, KD], F8, name="wk_sb")
    wv_sb = wp.tile([P, NCO, VD], CD, name="wv_sb")
    bv_sb = wp.tile([P, VD], F32, name="bv_sb")
    xstp = ctx.enter_context(tc.tile_pool(name="xstp", bufs=1, side="right"))
    xst8_sb = xstp.tile([P, NCO, SOWN], F8)
    xst16_sb = xstp.tile([P, NCO, SOWN], CD)

    # ---------------- phase A: qt8 = fp8 DoubleRow proj of q --------------
    # ci2 (contraction pairs) OUTER with 4 live PSUM banks per ko group,
    # so the first matmul depends only on the first ci row-pair
    # (~0.8 MB of DMA); weights/x stream in behind the compute.
    qtp = ctx.enter_context(tc.tile_pool(name="qtp", bufs=1))
    qt = qtp.tile([P, NKO, T], F8, tag="qt")
    xtp_cm = tc.tile_pool(name="xtp", bufs=1, side="right")
    xtp = xtp_cm.__enter__()
    xt_sb = xtp.tile([P, NCO, T], F8)
    # HAM warmup: keep the PE busy on dummy fp8 matmuls through the DMA
    # head so the clock gate is already 8/8 when phase A's matmuls start
    # (cold start otherwise runs the first ~4.5us of real work at 1.2GHz).
    warm = const.tile([P, SB], F8, name="warm")
    nc.vector.memset(warm[:], 0)
    wps = psum.tile([P, SB], F32, tag="ps", name="warmps")
    for _ in range(20):
        nc.tensor.matmul(wps[:], warm[:, :P], warm[:], start=True, stop=True)
    for ci in range(NCO):
        eng, eng2 = ((nc.sync, nc.gpsimd), (nc.gpsimd, nc.sync))[ci % 2]
        eng.dma_start(wq_sb[:, ci, :], wq8[:, ci, :])
        # halves so the first accumulation group unblocks sooner
        eng2.dma_start(xt_sb[:, ci, : T // 2], xt8[:, ci, : T // 2])
        eng2.dma_start(xt_sb[:, ci, T // 2 :], xt8[:, ci, T // 2 :])
    QDS = QS / (XS * WS)
    for ko2 in range(NKO // 2):
        for kk in range(2):
            ko = 2 * ko2 + kk
            pss = [
                psum.tile([P, SB], F32, tag="ps", name=f"psA{ko}_{j}")
                for j in range(4)
            ]
            for ci2 in range(NCO // 2):
                for sb in range(4):
                    nc.tensor.matmul(
                        pss[sb][:],
                        wq_sb[:, 2 * ci2 : 2 * ci2 + 2, ko * P : (ko + 1) * P],
                        xt_sb[:, 2 * ci2 : 2 * ci2 + 2, sb * SB : (sb + 1) * SB],
                        start=(ci2 == 0),
                        stop=(ci2 == NCO // 2 - 1),
                        perf_mode=DRM,
                    )
            for sb in range(4):
                nc.vector.scalar_tensor_tensor(
                    qt[:, ko, sb * SB : (sb + 1) * SB],
                    pss[sb][:],
                    QDS,
                    bq_sb[:, ko : ko + 1].to_broadcast((P, SB)),
                    MUL,
                    ADD,
                )

    xtp_cm.__exit__(None, None, None)

    # loads for phases B and D, queued behind phase A's streams
    for ci in range(NCO):
        nc.sync.dma_start(wk_sb[:, ci, :], wk8[:, ci, :])
    for sb in range(2):
        nc.sync.dma_start(
            xst8_sb[:, :, sb * SB : (sb + 1) * SB],
            xst8[:, :, sb * SB : (sb + 1) * SB],
        )
    for sb in range(2):
        nc.sync.dma_start(
            xst16_sb[:, :, sb * SB : (sb + 1) * SB],
            xst16[:, :, sb * SB : (sb + 1) * SB],
        )
    nc.sync.dma_start(wv_sb[:], wv16)
    nc.sync.dma_start(bv_sb[:], io["bv2"].ap())

    # ---------------- phase B: kt8_own, fp8 DoubleRow ----------------
    ktp = ctx.enter_context(tc.tile_pool(name="ktp", bufs=1))
    kt = ktp.tile([P, NKO, SOWN], F8, tag="kt")
    for ko2 in range(NKO // 2):
        pss = [
            psum.tile([P, SB], F32, tag="ps", name=f"psB{ko2}_{j}")
            for j in range(4)
        ]
        for ci2 in range(NCO // 2):
            for kk in range(2):
                ko = 2 * ko2 + kk
                for sb in range(2):
                    nc.tensor.matmul(
                        pss[kk * 2 + sb][:],
                        wk_sb[:, 2 * ci2 : 2 * ci2 + 2, ko * P : (ko + 1) * P],
                        xst8_sb[:, 2 * ci2 : 2 * ci2 + 2, sb * SB : (sb + 1) * SB],
                        start=(ci2 == 0),
                        stop=(ci2 == NCO // 2 - 1),
                        perf_mode=DRM,
                    )
        for kk in range(2):
            ko = 2 * ko2 + kk
            for sb in range(2):
                nc.vector.scalar_tensor_tensor(
                    kt[:, ko, sb * SB : (sb + 1) * SB],
                    pss[kk * 2 + sb][:],
                    QDS,
                    bk_sb[:, ko : ko + 1].to_broadcast((P, SB)),
                    MUL,
                    ADD,
                )

    # ------ phase C: et_i = exp((kt8_i.T @ qt8 + mask)/32768); scale ------
    etp = ctx.enter_context(tc.tile_pool(name="etp", bufs=1, side="right"))
    et = etp.tile([P, NI, T], CD, tag="et")
    et8 = etp.tile([P, NI, T], F8, tag="et8")
    dsum = const.tile([P, NI], F32, name="dsum")
    dinv = const.tile([P, NI], F32, name="dinv")
    dparts = const.tile([P, NI, 4], F32, name="dparts")
    for i in range(NI):
        tstart = 2 * i * P
        nchunk = 0
        t0 = tstart
        while t0 < T:
            w = min(SB, T - t0)
            ps = psum.tile([P, SB], F32, tag="ps")
            for k2 in range(NKO // 2):
                nc.tensor.matmul(
                    ps[:, :w],
                    kt[:, 2 * k2 : 2 * k2 + 2, i * P : (i + 1) * P],
                    qt[:, 2 * k2 : 2 * k2 + 2, t0 : t0 + w],
                    start=(k2 == 0),
                    stop=(k2 == NKO // 2 - 1),
                    perf_mode=DRM,
                )
            if nchunk == 0:
                nc.vector.tensor_add(ps[:, : 2 * P], ps[:, : 2 * P], mask_sb[:])
            nc.scalar.activation(
                et[:, i, t0 : t0 + w],
                ps[:, :w],
                mybir.ActivationFunctionType.Exp,
                scale=1.0 / (SQRT_K * QS * QS),
                accum_out=dparts[:, i, nchunk : nchunk + 1],
            )
            t0 += w
            nchunk += 1
        nc.vector.tensor_copy(dsum[:, i : i + 1], dparts[:, i, 0:1])
        for c in range(1, nchunk):
            nc.vector.tensor_add(
                dsum[:, i : i + 1], dsum[:, i : i + 1], dparts[:, i, c : c + 1]
            )
        nc.vector.reciprocal(dinv[:, i : i + 1], dsum[:, i : i + 1])
        # normalize on DVE, not ACT: phase C is ACT-throughput-limited
        # otherwise (exp + normalize both on ACT stall the PSUM banks).
        nc.vector.tensor_scalar_mul(
            et[:, i, tstart:], et[:, i, tstart:], dinv[:, i : i + 1]
        )

    # ---------------- phase D: v_own = xst16.T @ wv16 + bv (bf16) ---------
    # v16 carries v*4096 (exact pow2 in bf16) so phase E can mix bf16
    # near-diagonal matmuls with fp8 far ones ((p*128)*(v*32) = p*v*4096)
    # in one PSUM accumulation; the E drain folds 1/4096 back out.
    # bv_sb is host-prescaled by 4096 to match.
    vop = ctx.enter_context(tc.tile_pool(name="vop", bufs=1))
    v_own = vop.tile([P, NI, VD], CD)
    v8 = vop.tile([P, NI, VD], F8)
    for jl in range(NI):
        pss = [
            psum.tile([P, SB], F32, tag="ps", name=f"psD{jl}_{vb}")
            for vb in range(VD // SB)
        ]
        for ci in range(NCO):
            for vb in range(VD // SB):
                nc.tensor.matmul(
                    pss[vb][:],
                    xst16_sb[:, ci, jl * P : (jl + 1) * P],
                    wv_sb[:, ci, vb * SB : (vb + 1) * SB],
                    start=(ci == 0),
                    stop=(ci == NCO - 1),
                )
        for vb in range(VD // SB):
            nc.vector.scalar_tensor_tensor(
                v_own[:, jl, vb * SB : (vb + 1) * SB],
                pss[vb][:],
                4096.0,
                bv_sb[:, vb * SB : (vb + 1) * SB],
                MUL,
                ADD,
            )
        for vb in range(VD // SB):
            nc.scalar.mul(
                v8[:, jl, vb * SB : (vb + 1) * SB],
                v_own[:, jl, vb * SB : (vb + 1) * SB],
                2.0**-7,
            )

    # fp8 copies of the normalized probs (x128) for phase E's far blocks:
    # block i feeds groups g >= 2i+2 in fp8, i.e. t >= 2(i+1)P. Issued
    # HERE (after D) so they don't sit between exp ops in ACT's FIFO
    # during phase C, where their cross-engine dependency on the DVE
    # normalize would stall the PSUM drain chain.
    for i in range(NI - 1):
        nc.scalar.mul(
            et8[:, i, (2 * i + 2) * P :], et[:, i, (2 * i + 2) * P :], 128.0
        )

    # ------------- phase E: read_partial = sum_i et_i.T @ v_i -------------
    # Far s-blocks (i <= ni-2) in fp8 DoubleRow (their p entries are tiny,
    # CPU-emulated cost ~1e-4 of relmax); the near-diagonal block (i=ni-1,
    # where p is large) stays bf16. All accumulate p*v*4096 in one PSUM.
    read_out = io["read_out"].ap()
    with tc.tile_pool(name="rout", bufs=8) as rout:
        for g in range(T // P):
            ni = g // 2 + 1
            m = ni - 1
            pss = [
                psum.tile([P, SB], F32, tag="ps", name=f"psE{g}_{vb}")
                for vb in range(VD // SB)
            ]
            for a in range(m // 2):
                for vb in range(VD // SB):
                    nc.tensor.matmul(
                        pss[vb][:],
                        et8[:, 2 * a : 2 * a + 2, g * P : (g + 1) * P],
                        v8[:, 2 * a : 2 * a + 2, vb * SB : (vb + 1) * SB],
                        start=(a == 0),
                        stop=False,
                        perf_mode=DRM,
                    )
            if m % 2:
                for vb in range(VD // SB):
                    nc.tensor.matmul(
                        pss[vb][:],
                        et8[:, m - 1, g * P : (g + 1) * P],
                        v8[:, m - 1, vb * SB : (vb + 1) * SB],
                        start=(m // 2 == 0),
                        stop=False,
                    )
            for vb in range(VD // SB):
                nc.tensor.matmul(
                    pss[vb][:],
                    et[:, ni - 1, g * P : (g + 1) * P],
                    v_own[:, ni - 1, vb * SB : (vb + 1) * SB],
                    start=(m == 0),
                    stop=True,
                )
            for vb in range(VD // SB):
                ro = rout.tile([P, SB], CD, tag="rout")
                j = 2 * g + vb
                if j % 2 == 0:
                    nc.scalar.mul(ro[:], pss[vb][:], 2.0**-12)
                else:
                    nc.vector.tensor_scalar_mul(ro[:], pss[vb][:], 2.0**-12)
                dma_eng = (nc.sync, nc.gpsimd, nc.scalar)[j % 3]
                dma_eng.dma_start(
                    read_out[g * P : (g + 1) * P, vb * SB : (vb + 1) * SB],
                    ro[:],
                )


def _build_nc(num_devices=8):
    nc = bacc.Bacc(
        "TRN2", target_bir_lowering=False, debug=False, num_devices=num_devices
    )
    io = {}
    io["xt8"] = nc.dram_tensor("xt8", [C, T], F8, kind="ExternalInput")
    io["xst8"] = nc.dram_tensor("xst8", [C, SOWN], F8, kind="ExternalInput")
    io["xst16"] = nc.dram_tensor("xst16", [C, SOWN], CD, kind="ExternalInput")
    io["wq8"] = nc.dram_tensor("wq8", [C, KD], F8, kind="ExternalInput")
    io["wk8"] = nc.dram_tensor("wk8", [C, KD], F8, kind="ExternalInput")
    io["wv16"] = nc.dram_tensor("wv16", [C, VD], CD, kind="ExternalInput")
    io["bk2"] = nc.dram_tensor("bk2", [P, NKO], F32, kind="ExternalInput")
    io["bq2"] = nc.dram_tensor("bq2", [P, NKO], F32, kind="ExternalInput")
    io["bv2"] = nc.dram_tensor("bv2", [P, VD], F32, kind="ExternalInput")
    io["maskbias"] = nc.dram_tensor(
        "maskbias", [P, 2 * P], F32, kind="ExternalInput"
    )
    io["read_out"] = nc.dram_tensor(
        "read_out", [T, VD], CD, kind="ExternalOutput"
    )
    with tile.TileContext(nc) as tc:
        attn_body(tc, io)
    nc.compile()
    return nc


def _own_cols(h):
    idx = []
    for i in range(NI):
        g = 2 * i + h
        idx.extend(range(g * P, (g + 1) * P))
    return np.array(idx)


def _make_in_maps(x, Wq, bq, Wk, bk, Wv, bv):
    import ml_dtypes

    bf16 = ml_dtypes.bfloat16
    f8 = ml_dtypes.float8_e4m3
    x = np.asarray(x, np.float32)
    xs = np.float32(XS)
    ws = np.float32(WS)
    qs = np.float32(QS)
    Wq8 = np.ascontiguousarray((np.asarray(Wq, np.float32) * ws).astype(f8))
    Wk8 = np.ascontiguousarray((np.asarray(Wk, np.float32) * ws).astype(f8))
    Wv16 = np.ascontiguousarray(np.asarray(Wv, np.float32).astype(bf16))
    bq, bk, bv = (np.asarray(v, np.float32) for v in (bq, bk, bv))

    # mask bias sits in the phase-C PSUM (logits * QS^2); it must stay
    # hugely negative after the exp scale 1/(32*QS*QS).
    NEG = np.float32(-3.4e13)
    sr = np.arange(P)[:, None]
    tcc = np.arange(P)[None, :]
    tri = np.where(tcc >= sr, 0.0, NEG).astype(np.float32)
    masks = {}
    for h in (0, 1):
        m = np.zeros((P, 2 * P), np.float32)
        if h == 0:
            m[:, :P] = tri
        else:
            m[:, :P] = NEG
            m[:, P:] = tri
        masks[h] = m

    bk2 = np.ascontiguousarray((bk * qs).reshape(NKO, P).T)
    bq2 = np.ascontiguousarray((bq * qs).reshape(NKO, P).T)
    # phase D/E run at v*4096 internally (see attn_body); bias matches
    bv2 = np.ascontiguousarray(np.broadcast_to(bv[None, :] * 4096.0, (P, VD)))

    in_maps = []
    for core in range(8):
        b, h = core // 2, core % 2
        xt_b = np.ascontiguousarray(x[b].T)
        cols = _own_cols(h)
        xt8_b = (xt_b * xs).astype(f8)
        in_maps.append(
            {
                "xt8": np.ascontiguousarray(xt8_b),
                "xst8": np.ascontiguousarray(xt8_b[:, cols]),
                "xst16": np.ascontiguousarray(xt_b[:, cols].astype(bf16)),
                "wq8": Wq8,
                "wk8": Wk8,
                "wv16": Wv16,
                "bk2": bk2,
                "bq2": bq2,
                "bv2": bv2,
                "maskbias": masks[h],
            }
        )
    return in_maps


def _assemble_output(x, results):
    x = np.asarray(x, np.float32)
    out = np.empty((x.shape[0], T, C + VD), np.float32)
    out[:, :, :C] = x
    for b in range(x.shape[0]):
        out[b, :, C:] = results[2 * b]["read_out"].astype(np.float32) + results[
            2 * b + 1
        ]["read_out"].astype(np.float32)
    return out


_NC_CACHE = None


def _build():
    global _NC_CACHE
    if _NC_CACHE is None:
        _NC_CACHE = _build_nc(num_devices=8)
    return _NC_CACHE


def kernel(x, Wq, bq, Wk, bk, Wv, bv):
    from concourse.bass_utils import run_bass_kernel_spmd

    nc = _build()
    in_maps = _make_in_maps(x, Wq, bq, Wk, bk, Wv, bv)
    res = run_bass_kernel_spmd(nc, in_maps, core_ids=list(range(8)))
    return _assemble_output(x, res.results)


# revision 36
# speedup vs baseline: 1.0283x; 1.0283x over previous
"""Trainium2 Bass kernel for nn_AttentionBlock (B=4, T=2048, C=K=V=1024).

Self-contained: builds one SPMD Bass/Tile program, runs it on 8 NeuronCores
via run_bass_kernel_spmd, and reassembles the full output on the host.

Math (matches the reference):
  q/k/v = x @ W + b ; logits[b,t,s] = q.k, causal mask s<=t ;
  probs = softmax(logits/sqrt(K), axis=t)   # over the QUERY axis
  read = probs @ v ; out = concat(x, read, axis=2)

Sharding (zero-collective): core = 2*b + h owns batch b and the interleaved
key/value tiles sigma = 2*i + h (interleaving balances the causal triangle).
Because the softmax normalizes over the query axis t and each core has ALL
queries for its own key columns, the softmax is fully core-local. Each core
computes q in full, k/v only for its own columns, exp-normalized scores
et[s_own, t], and an additive partial read_h[t, v] = et^T @ v_own. The host
sums the two partials per batch and concatenates x.

Numerics: the q/k path runs in fp8e4m3 with DoubleRow matmuls (2
contraction rows/cycle): x is pre-scaled x16, Wq/Wk x256 and quantized to
fp8 on the host; phase A/B accumulate in f32 PSUM and write qt/kt as
fp8(q*32) (drain scale 1/128, bias pre-scaled x32). Phase C contracts
qt8.kt8 with DoubleRow; its PSUM holds logits*1024, folded into the exp
scale 1/(32*1024). The v projection (phase D) stays bf16 (fp8 there
busts the 2e-2 gate: v rounding hits the read linearly, while q/k
rounding only perturbs the softmax exponent). Phase E mixes dtypes per
s-block: the near-diagonal block (large p) runs bf16, far blocks run
fp8 DoubleRow (p8 = fp8(p*128), v8 = fp8(v*32)) - their p entries are
tiny, measured ~1e-4 extra relmax. The bf16 leg carries v*4096 so both
legs accumulate p*v*4096 in one PSUM; the drain folds 2^-12 back out.
read_out is written bf16 and upcast on the host. CPU-emulated end-to-end
relmax 1.17e-2 (gate 2e-2); bf16-everywhere measures 2.5e-3 on silicon.

A HAM warmup (20 dummy fp8 matmuls on a zeroed tile) keeps the PE busy
through the input-DMA head so the clock gate is 8/8 when phase A starts.

Phases (single dense PE pipeline, no collectives):
  A. qt8 [k, 2048] = fp8((wq8.T @ xt8)/128 + bq*32)     DoubleRow fp8
     (contraction ci2-outer with 4 live PSUM banks per ko group, so
      the first matmul only waits on ~0.8 MB of DMA, not 3 MB)
  B. kt8_own [k, 1024] likewise                          DoubleRow fp8
  C. et_i = exp((kt8_i.T @ qt8 + mask)/32768), fused row-sum -> dinv;
     et_i *= dinv on DVE (bf16) + fp8 x128 copy on ACT   DoubleRow fp8
  D. v_own [1024, v] = (xst16.T @ wv16 + bv)*4096 (bf16) + fp8 /128 copy
  E. read_partial[t, v] = sum_i et_i.T @ v_own_i -> DRAM (bf16;
     far blocks fp8 DoubleRow, near-diagonal block bf16)
"""

from contextlib import ExitStack

import numpy as np

import concourse.mybir as mybir
import concourse.tile as tile
from concourse import bacc
from concourse._compat import with_exitstack

P = 128
B = 4
T = 2048
C = 1024
KD = 1024
VD = 1024
NCO = C // P
NKO = KD // P
NI = 8
SOWN = NI * P
SQRT_K = 32.0
F32 = mybir.dt.float32
CD = mybir.dt.bfloat16
F8 = mybir.dt.float8e4
# Host-side quantization scales (all powers of two, folded exactly):
#   x8 = fp8(x*XS), w8 = fp8(W*WS), qt8 = q*QS  (via drain scale QS/(XS*WS))
XS = 16.0
WS = 256.0
QS = 32.0
SB = 512


@with_exitstack
def attn_body(ctx: ExitStack, tc, io):
    nc = tc.nc
    DRM = mybir.MatmulPerfMode.DoubleRow
    MUL = mybir.AluOpType.mult
    ADD = mybir.AluOpType.add
    xt8 = io["xt8"].ap().rearrange("(co ci) t -> ci co t", ci=P)
    xst8 = io["xst8"].ap().rearrange("(co ci) t -> ci co t", ci=P)
    xst16 = io["xst16"].ap().rearrange("(co ci) t -> ci co t", ci=P)
    wq8 = io["wq8"].ap().rearrange("(co ci) k -> ci co k", ci=P)
    wk8 = io["wk8"].ap().rearrange("(co ci) k -> ci co k", ci=P)
    wv16 = io["wv16"].ap().rearrange("(co ci) k -> ci co k", ci=P)

    const = ctx.enter_context(tc.tile_pool(name="const", bufs=1))
    bk_sb = const.tile([P, NKO], F32)
    bq_sb = const.tile([P, NKO], F32)
    mask_sb = const.tile([P, 2 * P], F32)
    # const loads ride the scalar-engine DMA queue so they don't delay
    # the critical first wq/xt chunks on the sync/gpsimd queues
    nc.scalar.dma_start(bk_sb[:], io["bk2"].ap())
    nc.scalar.dma_start(bq_sb[:], io["bq2"].ap())
    nc.scalar.dma_start(mask_sb[:], io["maskbias"].ap())

    psum = ctx.enter_context(tc.tile_pool(name="psum", bufs=8, space="PSUM"))

    wp = ctx.enter_context(tc.tile_pool(name="wp", bufs=1, side="right"))
    wq_sb = wp.tile([P, NCO, KD], F8, name="wq_sb")
    wk_sb = wp.tile([P, NCO, KD], F8, name="wk_sb")
    wv_sb = wp.tile([P, NCO, VD], CD, name="wv_sb")
    bv_sb = wp.tile([P, VD], F32, name="bv_sb")
    xstp = ctx.enter_context(tc.tile_pool(name="xstp", bufs=1, side="right"))
    xst8_sb = xstp.tile([P, NCO, SOWN], F8)
    xst16_sb = xstp.tile([P, NCO, SOWN], CD)

    # HAM warmup: keep the PE busy on dummy fp8 matmuls through the DMA
    # head so the clock gate is already 8/8 when phase B's matmuls start
    # (cold start otherwise runs the first ~4.5us of real work at 1.2GHz).
    warm = const.tile([P, SB], F8, name="warm")
    nc.vector.memset(warm[:], 0)
    wps = psum.tile([P, SB], F32, tag="ps", name="warmps")
    for _ in range(16):
        nc.tensor.matmul(wps[:], warm[:, :P], warm[:], start=True, stop=True)

    qtp = ctx.enter_context(tc.tile_pool(name="qtp", bufs=1))
    qt = qtp.tile([P, NKO, T], F8, tag="qt")
    xtp_cm = tc.tile_pool(name="xtp", bufs=1, side="right")
    xtp = xtp_cm.__enter__()
    xt_sb = xtp.tile([P, NCO, T], F8)

    # loads: phase B's operands first (its first group needs only ~0.4 MB,
    # so compute starts earliest); phase A's 3 MB streams during B.
    for ci in range(NCO):
        eng, eng2 = ((nc.sync, nc.gpsimd), (nc.gpsimd, nc.sync))[ci % 2]
        eng.dma_start(wk_sb[:, ci, :], wk8[:, ci, :])
        eng2.dma_start(
            xst8_sb[:, ci, : SOWN // 2], xst8[:, ci, : SOWN // 2]
        )
        eng2.dma_start(
            xst8_sb[:, ci, SOWN // 2 :], xst8[:, ci, SOWN // 2 :]
        )
    for ci in range(NCO):
        eng, eng2 = ((nc.sync, nc.gpsimd), (nc.gpsimd, nc.sync))[ci % 2]
        eng.dma_start(wq_sb[:, ci, :], wq8[:, ci, :])
        eng2.dma_start(xt_sb[:, ci, : T // 2], xt8[:, ci, : T // 2])
        eng2.dma_start(xt_sb[:, ci, T // 2 :], xt8[:, ci, T // 2 :])
    for sb in range(2):
        nc.sync.dma_start(
            xst16_sb[:, :, sb * SB : (sb + 1) * SB],
            xst16[:, :, sb * SB : (sb + 1) * SB],
        )
    nc.sync.dma_start(wv_sb[:], wv16)
    nc.sync.dma_start(bv_sb[:], io["bv2"].ap())
    QDS = QS / (XS * WS)

    # ---------------- phase B: kt8_own, fp8 DoubleRow ----------------
    ktp = ctx.enter_context(tc.tile_pool(name="ktp", bufs=1))
    kt = ktp.tile([P, NKO, SOWN], F8, tag="kt")
    for ko2 in range(NKO // 2):
        pss = [
            psum.tile([P, SB], F32, tag="ps", name=f"psB{ko2}_{j}")
            for j in range(4)
        ]
        for ci2 in range(NCO // 2):
            for kk in range(2):
                ko = 2 * ko2 + kk
                for sb in range(2):
                    nc.tensor.matmul(
                        pss[kk * 2 + sb][:],
                        wk_sb[:, 2 * ci2 : 2 * ci2 + 2, ko * P : (ko + 1) * P],
                        xst8_sb[:, 2 * ci2 : 2 * ci2 + 2, sb * SB : (sb + 1) * SB],
                        start=(ci2 == 0),
                        stop=(ci2 == NCO // 2 - 1),
                        perf_mode=DRM,
                    )
        for kk in range(2):
            ko = 2 * ko2 + kk
            for sb in range(2):
                nc.vector.scalar_tensor_tensor(
                    kt[:, ko, sb * SB : (sb + 1) * SB],
                    pss[kk * 2 + sb][:],
                    QDS,
                    bk_sb[:, ko : ko + 1].to_broadcast((P, SB)),
                    MUL,
                    ADD,
                )

    # ---------------- phase A: qt8 = fp8 DoubleRow proj of q --------------
    # ci2 (contraction pairs) OUTER with 4 live PSUM banks per ko group.
    for ko2 in range(NKO // 2):
        for kk in range(2):
            ko = 2 * ko2 + kk
            pss = [
                psum.tile([P, SB], F32, tag="ps", name=f"psA{ko}_{j}")
                for j in range(4)
            ]
            for ci2 in range(NCO // 2):
                for sb in range(4):
                    nc.tensor.matmul(
                        pss[sb][:],
                        wq_sb[:, 2 * ci2 : 2 * ci2 + 2, ko * P : (ko + 1) * P],
                        xt_sb[:, 2 * ci2 : 2 * ci2 + 2, sb * SB : (sb + 1) * SB],
                        start=(ci2 == 0),
                        stop=(ci2 == NCO // 2 - 1),
                        perf_mode=DRM,
                    )
            for sb in range(4):
                nc.vector.scalar_tensor_tensor(
                    qt[:, ko, sb * SB : (sb + 1) * SB],
                    pss[sb][:],
                    QDS,
                    bq_sb[:, ko : ko + 1].to_broadcast((P, SB)),
                    MUL,
                    ADD,
                )

    xtp_cm.__exit__(None, None, None)

    # ------ phase C: et_i = exp((kt8_i.T @ qt8 + mask)/32768); scale ------
    etp = ctx.enter_context(tc.tile_pool(name="etp", bufs=1, side="right"))
    et = etp.tile([P, NI, T], CD, tag="et")
    et8 = etp.tile([P, NI, T], F8, tag="et8")
    dsum = const.tile([P, NI], F32, name="dsum")
    dinv = const.tile([P, NI], F32, name="dinv")
    dparts = const.tile([P, NI, 4], F32, name="dparts")
    for i in range(NI):
        tstart = 2 * i * P
        nchunk = 0
        t0 = tstart
        while t0 < T:
            w = min(SB, T - t0)
            ps = psum.tile([P, SB], F32, tag="ps")
            for k2 in range(NKO // 2):
                nc.tensor.matmul(
                    ps[:, :w],
                    kt[:, 2 * k2 : 2 * k2 + 2, i * P : (i + 1) * P],
                    qt[:, 2 * k2 : 2 * k2 + 2, t0 : t0 + w],
                    start=(k2 == 0),
                    stop=(k2 == NKO // 2 - 1),
                    perf_mode=DRM,
                )
            if nchunk == 0:
                nc.vector.tensor_add(ps[:, : 2 * P], ps[:, : 2 * P], mask_sb[:])
            nc.scalar.activation(
                et[:, i, t0 : t0 + w],
                ps[:, :w],
                mybir.ActivationFunctionType.Exp,
                scale=1.0 / (SQRT_K * QS * QS),
                accum_out=dparts[:, i, nchunk : nchunk + 1],
            )
            t0 += w
            nchunk += 1
        nc.vector.tensor_copy(dsum[:, i : i + 1], dparts[:, i, 0:1])
        for c in range(1, nchunk):
            nc.vector.tensor_add(
                dsum[:, i : i + 1], dsum[:, i : i + 1], dparts[:, i, c : c + 1]
            )
        nc.vector.reciprocal(dinv[:, i : i + 1], dsum[:, i : i + 1])
        # normalize on DVE, not ACT: phase C is ACT-throughput-limited
        # otherwise (exp + normalize both on ACT stall the PSUM banks).
        nc.vector.tensor_scalar_mul(
            et[:, i, tstart:], et[:, i, tstart:], dinv[:, i : i + 1]
        )

    # ---------------- phase D: v_own = xst16.T @ wv16 + bv (bf16) ---------
    # v16 carries v*4096 (exact pow2 in bf16) so phase E can mix bf16
    # near-diagonal matmuls with fp8 far ones ((p*128)*(v*32) = p*v*4096)
    # in one PSUM accumulation; the E drain folds 1/4096 back out.
    # bv_sb is host-prescaled by 4096 to match.
    vop = ctx.enter_context(tc.tile_pool(name="vop", bufs=1))
    v_own = vop.tile([P, NI, VD], CD)
    v8 = vop.tile([P, NI, VD], F8)
    for jl in range(NI):
        pss = [
            psum.tile([P, SB], F32, tag="ps", name=f"psD{jl}_{vb}")
            for vb in range(VD // SB)
        ]
        for ci in range(NCO):
            for vb in range(VD // SB):
                nc.tensor.matmul(
                    pss[vb][:],
                    xst16_sb[:, ci, jl * P : (jl + 1) * P],
                    wv_sb[:, ci, vb * SB : (vb + 1) * SB],
                    start=(ci == 0),
                    stop=(ci == NCO - 1),
                )
        for vb in range(VD // SB):
            nc.vector.scalar_tensor_tensor(
                v_own[:, jl, vb * SB : (vb + 1) * SB],
                pss[vb][:],
                4096.0,
                bv_sb[:, vb * SB : (vb + 1) * SB],
                MUL,
                ADD,
            )
        for vb in range(VD // SB):
            nc.scalar.mul(
                v8[:, jl, vb * SB : (vb + 1) * SB],
                v_own[:, jl, vb * SB : (vb + 1) * SB],
                2.0**-7,
            )

    # fp8 copies of the normalized probs (x128) for phase E's far blocks:
    # block i feeds groups g >= 2i+2 in fp8, i.e. t >= 2(i+1)P. Issued
    # HERE (after D) so they don't sit between exp ops in ACT's FIFO
    # during phase C, where their cross-engine dependency on the DVE
    # normalize would stall the PSUM drain chain.
    for i in range(NI - 1):
        nc.scalar.mul(
            et8[:, i, (2 * i + 2) * P :], et[:, i, (2 * i + 2) * P :], 128.0
        )

    # ------------- phase E: read_partial = sum_i et_i.T @ v_i -------------
    # Far s-blocks (i <= ni-2) in fp8 DoubleRow (their p entries are tiny,
    # CPU-emulated cost ~1e-4 of relmax); the near-diagonal block (i=ni-1,
    # where p is large) stays bf16. All accumulate p*v*4096 in one PSUM.
    read_out = io["read_out"].ap()
    with tc.tile_pool(name="rout", bufs=8) as rout:
        for g in range(T // P):
            ni = g // 2 + 1
            m = ni - 1
            pss = [
                psum.tile([P, SB], F32, tag="ps", name=f"psE{g}_{vb}")
                for vb in range(VD // SB)
            ]
            for a in range(m // 2):
                for vb in range(VD // SB):
                    nc.tensor.matmul(
                        pss[vb][:],
                        et8[:, 2 * a : 2 * a + 2, g * P : (g + 1) * P],
                        v8[:, 2 * a : 2 * a + 2, vb * SB : (vb + 1) * SB],
                        start=(a == 0),
                        stop=False,
                        perf_mode=DRM,
                    )
            if m % 2:
                for vb in range(VD // SB):
                    nc.tensor.matmul(
                        pss[vb][:],
                        et8[:, m - 1, g * P : (g + 1) * P],
                        v8[:, m - 1, vb * SB : (vb + 1) * SB],
                        start=(m // 2 == 0),
                        stop=False,
                    )
            for vb in range(VD // SB):
                nc.tensor.matmul(
                    pss[vb][:],
                    et[:, ni - 1, g * P : (g + 1) * P],
                    v_own[:, ni - 1, vb * SB : (vb + 1) * SB],
                    start=(m == 0),
                    stop=True,
                )
            for vb in range(VD // SB):
                ro = rout.tile([P, SB], CD, tag="rout")
                j = 2 * g + vb
                if j % 2 == 0:
                    nc.scalar.mul(ro[:], pss[vb][:], 2.0**-12)
                else:
                    nc.vector.tensor_scalar_mul(ro[:], pss[vb][:], 2.0**-12)
                dma_eng = (nc.sync, nc.gpsimd, nc.scalar)[j % 3]
                dma_eng.dma_start(
                    read_out[g * P : (g + 1) * P, vb * SB : (vb + 1) * SB],
                    ro[:],
                )


def _build_nc(num_devices=8):
    nc = bacc.Bacc(
        "TRN2", target_bir_lowering=False, debug=False, num_devices=num_devices
    )
    io = {}
    io["xt8"] = nc.dram_tensor("xt8", [C, T], F8, kind="ExternalInput")
    io["xst8"] = nc.dram_tensor("xst8", [C, SOWN], F8, kind="ExternalInput")
    io["xst16"] = nc.dram_tensor("xst16", [C, SOWN], CD, kind="ExternalInput")
    io["wq8"] = nc.dram_tensor("wq8", [C, KD], F8, kind="ExternalInput")
    io["wk8"] = nc.dram_tensor("wk8", [C, KD], F8, kind="ExternalInput")
    io["wv16"] = nc.dram_tensor("wv16", [C, VD], CD, kind="ExternalInput")
    io["bk2"] = nc.dram_tensor("bk2", [P, NKO], F32, kind="ExternalInput")
    io["bq2"] = nc.dram_tensor("bq2", [P, NKO], F32, kind="ExternalInput")
    io["bv2"] = nc.dram_tensor("bv2", [P, VD], F32, kind="ExternalInput")
    io["maskbias"] = nc.dram_tensor(
        "maskbias", [P, 2 * P], F32, kind="ExternalInput"
    )
    io["read_out"] = nc.dram_tensor(
        "read_out", [T, VD], CD, kind="ExternalOutput"
    )
    with tile.TileContext(nc) as tc:
        attn_body(tc, io)
    nc.compile()
    return nc


def _own_cols(h):
    idx = []
    for i in range(NI):
        g = 2 * i + h
        idx.extend(range(g * P, (g + 1) * P))
    return np.array(idx)


def _make_in_maps(x, Wq, bq, Wk, bk, Wv, bv):
    import ml_dtypes

    bf16 = ml_dtypes.bfloat16
    f8 = ml_dtypes.float8_e4m3
    x = np.asarray(x, np.float32)
    xs = np.float32(XS)
    ws = np.float32(WS)
    qs = np.float32(QS)
    Wq8 = np.ascontiguousarray((np.asarray(Wq, np.float32) * ws).astype(f8))
    Wk8 = np.ascontiguousarray((np.asarray(Wk, np.float32) * ws).astype(f8))
    Wv16 = np.ascontiguousarray(np.asarray(Wv, np.float32).astype(bf16))
    bq, bk, bv = (np.asarray(v, np.float32) for v in (bq, bk, bv))

    # mask bias sits in the phase-C PSUM (logits * QS^2); it must stay
    # hugely negative after the exp scale 1/(32*QS*QS).
    NEG = np.float32(-3.4e13)
    sr = np.arange(P)[:, None]
    tcc = np.arange(P)[None, :]
    tri = np.where(tcc >= sr, 0.0, NEG).astype(np.float32)
    masks = {}
    for h in (0, 1):
        m = np.zeros((P, 2 * P), np.float32)
        if h == 0:
            m[:, :P] = tri
        else:
            m[:, :P] = NEG
            m[:, P:] = tri
        masks[h] = m

    bk2 = np.ascontiguousarray((bk * qs).reshape(NKO, P).T)
    bq2 = np.ascontiguousarray((bq * qs).reshape(NKO, P).T)
    # phase D/E run at v*4096 internally (see attn_body); bias matches
    bv2 = np.ascontiguousarray(np.broadcast_to(bv[None, :] * 4096.0, (P, VD)))

    in_maps = []
    for core in range(8):
        b, h = core // 2, core % 2
        xt_b = np.ascontiguousarray(x[b].T)
        cols = _own_cols(h)
        xt8_b = (xt_b * xs).astype(f8)
        in_maps.append(
            {
                "xt8": np.ascontiguousarray(xt8_b),
                "xst8": np.ascontiguousarray(xt8_b[:, cols]),
                "xst16": np.ascontiguousarray(xt_b[:, cols].astype(bf16)),
                "wq8": Wq8,
                "wk8": Wk8,
                "wv16": Wv16,
                "bk2": bk2,
                "bq2": bq2,
                "bv2": bv2,
                "maskbias": masks[h],
            }
        )
    return in_maps


def _assemble_output(x, results):
    x = np.asarray(x, np.float32)
    out = np.empty((x.shape[0], T, C + VD), np.float32)
    out[:, :, :C] = x
    for b in range(x.shape[0]):
        out[b, :, C:] = results[2 * b]["read_out"].astype(np.float32) + results[
            2 * b + 1
        ]["read_out"].astype(np.float32)
    return out


_NC_CACHE = None


def _build():
    global _NC_CACHE
    if _NC_CACHE is None:
        _NC_CACHE = _build_nc(num_devices=8)
    return _NC_CACHE


def kernel(x, Wq, bq, Wk, bk, Wv, bv):
    from concourse.bass_utils import run_bass_kernel_spmd

    nc = _build()
    in_maps = _make_in_maps(x, Wq, bq, Wk, bk, Wv, bv)
    res = run_bass_kernel_spmd(nc, in_maps, core_ids=list(range(8)))
    return _assemble_output(x, res.results)
